# revision 23
# baseline (speedup 1.0000x reference)
"""ExpertGraphConv Trainium2 kernel.

Full inputs in, full output out. Shards batch dim (B=8) across 8 NeuronCores;
params replicated. Each core processes 2048 tokens x 8 experts = 16384 rows.

Math per (token t, expert i):
  adj = sigmoid(adjacency_logits); wa, wb = w_msg[:D], w_msg[D:]
  a[t,i] = x[t,i] . wa ; b[t,j] = x[t,j] . wb
  strength[t,i,j] = adj[i,j] * sigmoid(a[t,i]+b[t,j]+b_msg) * (i != j)
  msg[t,i] = sum_j strength[t,i,j] x[t,j]
  out = gelu(msg @ Wn^T + x @ Ws^T + bn + bs)       (exact erf gelu)

Device mapping (per 128-row chunk = 16 tokens x 8 experts):
  - a/b via DVE scalar_tensor_tensor (fused mult + row-sum) against
    broadcast wa/wb, accumulated into one static [128,6] tile laid out
    as [b | 1 | bmsg | 1 | a | 1]
  - ONE f32 PE transpose of that tile -> [6,128] PSUM, one DVE copy to
    bf16 SBUF; outer sum a[i]+b[j]+bmsg via one bf16 K=3 matmul
    (lhsT=rows 0..3, rhs=rows 3..6) -> [128,128] in (t,j) x (t,i) layout
  - sigmoid via tanh identity on ACT (bf16 out), strength = (th+1) *
    0.5*adj*mask via DVE STT into the left half of a static [Sblk | I]
    rhs tile (f32)
  - one fp32r matmul per 128-d-chunk with rhs=[Sblk | I128] yields msg^T
    and x^T together in PSUM (256-wide output keeps fp32r at 1 cyc/row)
  - ACT copies PSUM -> bf16 zt; 8 accumulating bf16 matmuls against
    pre-transposed full Wn / Ws give u = msg@Wn^T + x@Ws^T in PSUM
  - Pool (gpsimd) adds the replicated bias in-place in PSUM
  - single exact-Gelu ACT op (gelu_and_others table also holds tanh:
    no table reloads in steady state) -> f32 SBUF -> DMA out
"""

import math
from contextlib import ExitStack

import numpy as np

import concourse.bacc as bacc
import concourse.mybir as mybir
import concourse.tile as tile
from concourse import bass_utils
from concourse.masks import make_identity

F32 = mybir.dt.float32
F32R = mybir.dt.float32r
BF16 = mybir.dt.bfloat16
AF = mybir.ActivationFunctionType
OP = mybir.AluOpType

B, L, E, D = 8, 2048, 8, 512
N_CORES = 8
P = 128
ROWS_PER_CORE = (B // N_CORES) * L * E  # 16384
NSC = 8  # rotation depth of the static per-chunk rhs/ab buffers


def build_nc(n_rows=ROWS_PER_CORE, gelu_tanh_standin=False):
    assert n_rows % P == 0
    n_chunks = n_rows // P
    nd = D // P  # 4 d-chunks

    nc = bacc.Bacc(
        "TRN2", target_bir_lowering=False, debug=False, num_devices=N_CORES
    )

    x_dram = nc.dram_tensor("expert_features", [n_rows, D], F32, kind="ExternalInput").ap()
    wn_dram = nc.dram_tensor("W_neighbor", [D, D], F32, kind="ExternalInput").ap()
    bn_dram = nc.dram_tensor("b_neighbor", [1, D], F32, kind="ExternalInput").ap()
    ws_dram = nc.dram_tensor("W_self", [D, D], F32, kind="ExternalInput").ap()
    bs_dram = nc.dram_tensor("b_self", [1, D], F32, kind="ExternalInput").ap()
    wmsg_dram = nc.dram_tensor("w_msg", [1, 2 * D], F32, kind="ExternalInput").ap()
    bmsg_dram = nc.dram_tensor("b_msg", [1, 1], F32, kind="ExternalInput").ap()
    adj_dram = nc.dram_tensor("adjacency_logits", [E, E], F32, kind="ExternalInput").ap()
    out_dram = nc.dram_tensor("out", [n_rows, D], F32, kind="ExternalOutput").ap()

    def r(ap):
        return ap.bitcast(F32R)

    with tile.TileContext(nc) as tc, ExitStack() as ctx:
        # ---- static SBUF tensors ----
        I128 = nc.alloc_sbuf_tensor("c_I128", [P, P], F32).ap()
        wa_rep = nc.alloc_sbuf_tensor("c_wa_rep", [P, D], F32).ap()
        wb_rep = nc.alloc_sbuf_tensor("c_wb_rep", [P, D], F32).ap()
        hadj_blk = nc.alloc_sbuf_tensor("c_hadj_blk", [P, P], BF16).ap()
        ones_row = nc.alloc_sbuf_tensor("c_ones_row", [1, P], F32).ap()
        ones_bf = nc.alloc_sbuf_tensor("c_ones_bf", [1, P], BF16).ap()
        bias_bf = nc.alloc_sbuf_tensor("c_bias_bf", [1, D], BF16).ap()
        qmask8 = nc.alloc_sbuf_tensor("c_qmask8", [E, E], F32).ap()
        wnT = [nc.alloc_sbuf_tensor(f"c_wnT{c}", [P, D], BF16).ap() for c in range(nd)]
        wsT = [nc.alloc_sbuf_tensor(f"c_wsT{c}", [P, D], BF16).ap() for c in range(nd)]
        # strength rhs [Sblk | I], f32 (stage-1 runs in fp32r at 1 cyc/row
        # thanks to its 256-wide outputs)
        scat = [
            nc.alloc_sbuf_tensor(f"c_scat{i}", [P, 2 * P], F32).ap()
            for i in range(NSC)
        ]
        # cols 0..3 = [b | 1 | bmsg], cols 32..35 = [1 | a | 1]; after PE
        # transpose the triples land at partitions 0 and 32 (DVE PSUM reads
        # must be 32-partition aligned). Cols 0/33 written per chunk.
        ABW = 35
        ab_s = [
            nc.alloc_sbuf_tensor(f"c_ab{i}", [P, ABW], F32).ap() for i in range(NSC)
        ]

        make_identity(nc, I128)
        nc.gpsimd.memset(ones_row, 1.0)
        nc.gpsimd.memset(ones_bf, 1.0)
        for t_ in ab_s:
            nc.gpsimd.memset(t_[:, 1:2], 1.0)
            nc.gpsimd.memset(t_[:, 3:32], 0.0)
            nc.gpsimd.memset(t_[:, 32:33], 1.0)
            nc.gpsimd.memset(t_[:, 34:35], 1.0)
        # qmask8: 0.25 off-diagonal, 0 on diagonal (two 0.5s: one per
        # tanh->sigmoid identity, for adj and for content)
        nc.gpsimd.memset(qmask8, 0.25)
        nc.gpsimd.affine_select(
            out=qmask8,
            in_=qmask8,
            compare_op=OP.not_equal,
            fill=0.0,
            base=0,
            pattern=[[-1, E]],
            channel_multiplier=1,
        )
        nc.gpsimd.memset(hadj_blk, 0.0)
        for s in scat:
            nc.gpsimd.tensor_copy(r(s[:, P : 2 * P]), I128)

        # ---- setup: params -> transposed/broadcast SBUF form ----
        with (
            tc.tile_pool(name="su", bufs=2) as su,
            tc.tile_pool(name="su_ps", bufs=2, space="PSUM") as sups,
        ):
            for mdram, dst in ((wn_dram, wnT), (ws_dram, wsT)):
                for oc in range(nd):
                    nat = su.tile([P, D], F32, tag="wnat")
                    nc.sync.dma_start(nat[:], mdram[oc * P : (oc + 1) * P, :])
                    for dc in range(nd):
                        ps = sups.tile([P, P], F32, tag="tps")
                        nc.tensor.transpose(
                            ps[:], nat[:, dc * P : (dc + 1) * P], I128
                        )
                        nc.scalar.copy(dst[dc][:, oc * P : (oc + 1) * P], ps[:])

            wmsg_sb = su.tile([1, 2 * D], F32)
            nc.sync.dma_start(wmsg_sb[:], wmsg_dram[:])
            psa = sups.tile([P, D], F32, tag="bps")
            nc.tensor.matmul(psa[:], lhsT=ones_row, rhs=wmsg_sb[:, 0:D])
            nc.vector.tensor_copy(wa_rep, psa[:])
            psb = sups.tile([P, D], F32, tag="bps")
            nc.tensor.matmul(psb[:], lhsT=ones_row, rhs=wmsg_sb[:, D : 2 * D])
            nc.vector.tensor_copy(wb_rep, psb[:])

            bmsg_sb = su.tile([1, 1], F32)
            nc.sync.dma_start(bmsg_sb[:], bmsg_dram[:])
            psm = sups.tile([P, 1], F32, tag="sps")
            nc.tensor.matmul(psm[:], lhsT=ones_row, rhs=bmsg_sb[:])
            for t_ in ab_s:
                nc.vector.tensor_copy(t_[:, 2:3], psm[:])

            bn_sb = su.tile([1, D], F32)
            bs_sb = su.tile([1, D], F32)
            nc.sync.dma_start(bn_sb[:], bn_dram[:])
            nc.sync.dma_start(bs_sb[:], bs_dram[:])
            bsum = su.tile([1, D], F32)
            nc.vector.tensor_add(bsum[:], bn_sb[:], bs_sb[:])
            nc.vector.tensor_copy(bias_bf, bsum[:])

            # adjacency: need adj^T blocks. Load natural, PE-transpose 8x8.
            adjn = su.tile([E, E], F32)
            nc.sync.dma_start(adjn[:], adj_dram[:])
            psd = sups.tile([E, E], F32, tag="sps")
            nc.tensor.transpose(psd[:], adjn[:], I128[0:E, 0:E])
            adjT = su.tile([E, E], F32)
            nc.vector.tensor_copy(adjT[:], psd[:])
            t8 = su.tile([E, E], F32)
            nc.scalar.activation(t8[:], adjT[:], AF.Tanh, scale=0.5)
            h8 = su.tile([E, E], BF16)
            # 0.5*sigmoid(adjL[i,j]) masked: (tanh+1) * qmask8
            nc.vector.scalar_tensor_tensor(
                out=h8[:], in0=t8[:], scalar=1.0, in1=qmask8,
                op0=OP.add, op1=OP.mult,
            )
            # DMA (not DVE): block starts are not 32-aligned partitions
            for t in range(P // E):
                nc.sync.dma_start(
                    hadj_blk[t * E : (t + 1) * E, t * E : (t + 1) * E], h8[:]
                )

        # ---- main loop ----
        with (
            tc.tile_pool(name="xp", bufs=8) as xp,
            tc.tile_pool(name="sc", bufs=2) as scp,
            tc.tile_pool(name="small", bufs=4) as smp,
            tc.tile_pool(name="mid", bufs=3) as midp,
            tc.tile_pool(name="ztp", bufs=3) as ztp,
            tc.tile_pool(name="op", bufs=6) as op_,
            tc.tile_pool(name="ps_s", bufs=2, space="PSUM") as ps_s,
            tc.tile_pool(name="ps_c", bufs=2, space="PSUM") as ps_c,
            tc.tile_pool(name="ps_b", bufs=2, space="PSUM") as ps_b,
        ):
            state = {}

            def em_dma(c):
                rows = slice(c * P, (c + 1) * P)
                xin = xp.tile([P, D], F32, tag="xin", name=f"xin{c}")
                nc.sync.dma_start(r(xin[:]), r(x_dram[rows, :]))
                state[("xin", c)] = xin

            def em_a(c):
                # a = x.wa via DVE fused mult+row-sum into the static ab tile
                xin = state[("xin", c)]
                scr = scp.tile([P, D], F32, tag="scra", name=f"scra{c}")
                nc.vector.scalar_tensor_tensor(
                    out=scr[:], in0=xin[:], scalar=0.0, in1=wa_rep,
                    op0=OP.bypass, op1=OP.mult,
                    accum_out=ab_s[c % NSC][:, 33:34],
                )

            def em_b(c):
                xin = state[("xin", c)]
                scr2 = scp.tile([P, D], F32, tag="scrb", name=f"scrb{c}")
                nc.vector.scalar_tensor_tensor(
                    out=scr2[:], in0=xin[:], scalar=0.0, in1=wb_rep,
                    op0=OP.bypass, op1=OP.mult,
                    accum_out=ab_s[c % NSC][:, 0:1],
                )

            def em_abT(c):
                # one PSUM bank hosts both the ab-transpose and the outer sum
                sp = ps_s.tile([P, 2 * P], F32, tag="sp", name=f"sp{c}")
                nc.tensor.transpose(sp[0:ABW, 0:P], ab_s[c % NSC][:], I128)
                state[("sp", c)] = sp

            def em_copies(c):
                sp = state[("sp", c)]
                abT = sp[0:ABW, 0:P]
                lhsB = smp.tile([3, P], BF16, tag="lhsB", name=f"lhsB{c}")
                rhsA = smp.tile([3, P], BF16, tag="rhsA", name=f"rhsA{c}")
                nc.vector.tensor_copy(lhsB[:], abT[0:3, :])
                nc.vector.tensor_copy(rhsA[:], abT[32:35, :])
                state[("lr", c)] = (lhsB, rhsA)

            def em_outer(c):
                sp = state[("sp", c)]
                lhsB, rhsA = state.pop(("lr", c))
                nc.tensor.matmul(sp[:, P : 2 * P], lhsT=lhsB[:], rhs=rhsA[:])

            def em_tanh(c):
                sp = state.pop(("sp", c))
                th = midp.tile([P, P], BF16, tag="th", name=f"th{c}")
                nc.scalar.activation(th[:], sp[:, P : 2 * P], AF.Tanh, scale=0.5)
                state[("th", c)] = th

            def em_strength(c):
                th = state.pop(("th", c))
                sb = scat[c % NSC]
                # strength = (tanh+1) * 0.25*adjmask into left half of [Sblk|I]
                nc.vector.scalar_tensor_tensor(
                    out=r(sb[:, 0:P]), in0=th[:], scalar=1.0, in1=hadj_blk,
                    op0=OP.add, op1=OP.mult,
                )

            def em_stage1(c):
                # msg^T and x^T together: one fp32r matmul per d-chunk
                xin = state.pop(("xin", c))
                sb = scat[c % NSC]
                cmb = ps_c.tile([P, 2 * P * nd], F32, tag="cmb", name=f"cmb{c}")
                for dc in range(nd):
                    nc.tensor.matmul(
                        cmb[:, 2 * P * dc : 2 * P * (dc + 1)],
                        lhsT=r(xin[:, dc * P : (dc + 1) * P]),
                        rhs=r(sb[:]),
                    )
                state[("cmb", c)] = cmb

            def em_ztcopy(c):
                cmb = state.pop(("cmb", c))
                zt = ztp.tile([P, 2 * P * nd], BF16, tag="zt", name=f"zt{c}")
                nc.scalar.copy(zt[:], cmb[:])
                state[("zt", c)] = zt

            def em_stage2(c):
                zt = state.pop(("zt", c))
                big = ps_b.tile([P, D], F32, tag="big", name=f"big{c}")
                # bias first: depends only on static operands, so the PE can
                # issue it into any dependency gap
                nc.tensor.matmul(
                    big[:], lhsT=ones_bf, rhs=bias_bf, start=True, stop=False,
                )
                for dc in range(nd):
                    nc.tensor.matmul(
                        big[:],
                        lhsT=zt[:, 2 * P * dc : 2 * P * dc + P],
                        rhs=wnT[dc],
                        start=False,
                        stop=False,
                    )
                for dc in range(nd):
                    nc.tensor.matmul(
                        big[:],
                        lhsT=zt[:, 2 * P * dc + P : 2 * P * (dc + 1)],
                        rhs=wsT[dc],
                        start=False,
                        stop=(dc == nd - 1),
                    )
                state[("big", c)] = big

            def em_out(c):
                big = state.pop(("big", c))
                osb = op_.tile([P, D], F32, tag="osb", name=f"osb{c}")
                nc.scalar.activation(
                    osb[:], big[:],
                    AF.Tanh if gelu_tanh_standin else AF.Gelu, scale=1.0,
                )
                nc.sync.dma_start(out_dram[c * P : (c + 1) * P, :], osb[:])

            # Software pipeline: the ab->outer->tanh->strength chain for
            # chunk k+2 runs two iterations ahead of its stage-1, so its
            # cross-engine latency never blocks the PE. Per-engine orders:
            #   PE : abT(k+2), stage1(k+1), outer(k+2), bias+stage2(k)
            #   DVE: copies(k+2), a/b(k+3), strength(k+2)
            #   ACT: tanh(k+2), ztcopy(k+1), gelu(k)
            n = n_chunks

            def emit(k):
                def ok(c):
                    return 0 <= c < n
                if ok(k + 3):
                    em_dma(k + 3)
                    em_a(k + 3)
                if ok(k + 2):
                    em_abT(k + 2)
                if ok(k + 1):
                    em_stage1(k + 1)
                    em_ztcopy(k + 1)
                if ok(k + 2):
                    em_copies(k + 2)
                    em_outer(k + 2)
                    em_tanh(k + 2)
                if ok(k + 3):
                    em_b(k + 3)
                if ok(k + 2):
                    em_strength(k + 2)
                if ok(k):
                    em_stage2(k)
                    em_out(k)

            em_dma(0)
            em_a(0)
            em_b(0)
            for k in range(-2, n):
                emit(k)

    nc.compile()
    return nc


_CACHE = {}


def _get_nc():
    if "nc" not in _CACHE:
        _CACHE["nc"] = build_nc()
    return _CACHE["nc"]


def _make_in_maps(inputs):
    x = np.ascontiguousarray(np.asarray(inputs["expert_features"], np.float32))
    assert x.shape == (B, L, E, D)
    shards = x.reshape(N_CORES, ROWS_PER_CORE, D)
    params = {
        "W_neighbor": np.ascontiguousarray(np.asarray(inputs["W_neighbor"], np.float32)),
        "b_neighbor": np.asarray(inputs["b_neighbor"], np.float32).reshape(1, D),
        "W_self": np.ascontiguousarray(np.asarray(inputs["W_self"], np.float32)),
        "b_self": np.asarray(inputs["b_self"], np.float32).reshape(1, D),
        "w_msg": np.asarray(inputs["w_msg"], np.float32).reshape(1, 2 * D),
        "b_msg": np.asarray(inputs["b_msg"], np.float32).reshape(1, 1),
        "adjacency_logits": np.ascontiguousarray(
            np.asarray(inputs["adjacency_logits"], np.float32)
        ),
    }
    return [dict(expert_features=shards[c], **params) for c in range(N_CORES)]


def _run(inputs, trace=False):
    nc = _get_nc()
    in_maps = _make_in_maps(inputs)
    res = bass_utils.run_bass_kernel_spmd(
        nc, in_maps, core_ids=list(range(N_CORES)), trace=trace
    )
    out = np.stack([res.results[c]["out"] for c in range(N_CORES)], axis=0)
    return out.reshape(B, L, E, D), res


def kernel(**inputs):
    out, _ = _run(inputs, trace=False)
    return out
